# revision 1
# baseline (speedup 1.0000x reference)
"""Trainium2 kernel for nn_BatchedDTW.

The reference's banded-DTW recurrence is
    R[i, j] = D[i-1, j-1] + min(R[i-1, j-1], R[i-1, j])
whose predecessors both have row i-1, so i increments on every path step and
j never decreases. A path (0,0) -> (T,T) therefore takes exactly T steps and
must raise j by T, i.e. every step is diagonal, giving exactly
    R[T, T] = sum_t D[t, t] = sum_t ||x_t - y_t||_2
for any window >= 0 (the diagonal always satisfies |i-j| <= w).
So the whole problem collapses to
    out = mean_{b,n} sum_t ||X[b,t,n,:] - Y[b,t,n,:]||_2
       = (1/(B*N)) * sum over all (b,t,n) rows of sqrt(sum_c (X-Y)^2),
a pure streaming reduction over the flattened (B*T*N, C) rows, which we
shard contiguously across the 8 cores (row order is irrelevant to a sum).

Raw Bass (no Tile): this walrus build rejects instructions carrying more
than one sync wait, which Tile's tail drain needs, so semaphores are
explicit; every wait is its own instruction.

Engine layout (style="hyb", default):
  SP  : input chunk DMAs (HWDGE ring qSP); x|y packed per chunk so each
        chunk is ONE DMA -> one sem per consumer wait
  DVE : streaming chunks: sub (square runs on ACT, reduce back on DVE,
        keeping DVE at 2 passes, under the DMA stream); the LAST chunk
        runs sub/square/reduce entirely on DVE in program order, removing
        both cross-engine hops from the critical tail
  ACT : squares for streaming chunks; final sqrt (+free-dim accumulate in
        the same instruction); then issues the 512-B output DMA from its
        own HWDGE ring (qAct) and waits for its completion
Other styles kept for benching: "act" (square on ACT for all chunks),
"dve" (everything on DVE), "dual" (input DMAs split across both HWDGE
rings).
"""

from contextlib import ExitStack

import numpy as np

import concourse.bass as bass
import concourse.mybir as mybir
from concourse.bass_utils import run_bass_kernel_spmd

N_CORES = 8
P = 128                       # SBUF partitions
C = 32                        # channels per row (innermost axis of X/Y)
B, T, N = 4, 512, 64
ROWS_TOTAL = B * T * N        # 131072 rows of length C
ROWS_PER_CORE = ROWS_TOTAL // N_CORES     # 16384
F = ROWS_PER_CORE * C // P    # 4096 f32 per partition (16 KiB)
NCH = 8                       # input chunks -> 0.5 MiB DMAs
GT = F // C                   # 128 rows (length-C groups) per partition
# Default (graded) chunk-size vector, elems/partition per x (or y) half.
# Uniform 512-elem chunks (4 KiB descriptors, 0.5 MiB DMAs): a tiny final
# chunk was evaluated to shrink the post-stream DVE tail, but the last
# DMA's own fixed processing time (64 KiB is descriptor-dominated) delays
# the stream's last byte by about what the tail saves — not worth it.
CHUNK_SIZES = [F // NCH] * NCH
assert sum(CHUNK_SIZES) == F and all(s % C == 0 for s in CHUNK_SIZES)

_nc_cache = None
_last_results = None  # BassKernelResults from the most recent run (for benching)


def _build(nch=None, repeat=1, style="hyb"):
    """Build the per-core program.

    nch=None (the graded default) uses the CHUNK_SIZES vector with the hyb
    builder; an explicit nch gives uniform chunks (bench paths).
    repeat > 1 re-runs the whole pipeline on the same input (double-buffered
    SBUF, exact semaphore bookkeeping) purely so on-device time per pipeline
    iteration can be measured as a slope between two repeat counts; the
    graded kernel uses repeat=1.
    """
    assert style in ("dve", "act", "dual", "hyb", "hyd")
    if nch is None:
        assert style == "hyb"
        fcs = list(CHUNK_SIZES)
        nch = len(fcs)
    else:
        fcs = [F // nch] * nch
    fc = fcs[0]            # uniform-chunk styles use this
    gpc = fc // C          # rows per chunk per partition
    nbuf = 2 if repeat > 1 else 1
    nc = bass.Bass()
    f32 = mybir.dt.float32
    z_ext = nc.declare_dram_parameter("z", [P, 2 * F], f32, isOutput=False)
    out_ext = nc.declare_dram_parameter("out", [P, 1], f32, isOutput=True)

    with ExitStack() as ctx:
        zt = ctx.enter_context(nc.sbuf_tensor([P, nbuf * 2 * F], f32))
        df = ctx.enter_context(nc.sbuf_tensor([P, nbuf * F], f32))
        sq = ctx.enter_context(nc.sbuf_tensor([P, nbuf * F], f32))
        gs = ctx.enter_context(nc.sbuf_tensor([P, nbuf * GT], f32))
        dist = ctx.enter_context(nc.sbuf_tensor([P, nbuf * GT], f32))
        acc = ctx.enter_context(nc.sbuf_tensor([P, nbuf], f32))
        # One sem per in-flight input DMA: concurrent DMAs can complete out
        # of order, so cumulative waits on one shared sem would be racy.
        # Across repeats, sems are reused from a ring of KSETS sets with
        # cumulative (monotone) targets; the nbuf-deep buffer recycling
        # guarantees no two DMAs on the same sem are ever in flight at once.
        KSETS = min(repeat, 4)
        zsems = [ctx.enter_context(nc.semaphore(f"zsem{r}_{i}"))
                 for r in range(KSETS) for i in range(nch)]
        vsem = ctx.enter_context(nc.semaphore("vsem"))
        asem = ctx.enter_context(nc.semaphore("asem"))
        osem = ctx.enter_context(nc.semaphore("osem"))
        block = ctx.enter_context(nc.Block())

        def zs(r, ch):
            return zsems[(r % KSETS) * nch + ch]

        def z_done(r):  # zs(r, ch) value once rep r's chunk-ch DMA landed
            return 16 * (r // KSETS + 1)

        def zoff(r):  # free-dim base of rep r's zt buffer
            return (r % nbuf) * 2 * F

        def foff(r):
            return (r % nbuf) * F

        builder = {"dve": _build_dve, "act": _build_act,
                   "dual": _build_dual, "hyb": _build_hyb,
                   "hyd": _build_hyd}[style]
        builder(nc, block, repeat, nch, fc, gpc, nbuf,
                zt, df, sq, gs, dist, acc,
                z_ext, out_ext, zs, z_done, zoff, foff,
                vsem, asem, osem, fcs)
    return nc


def _build_dual(nc, block, repeat, nch, fc, gpc, nbuf,
                zt, df, sq, gs, dist, acc,
                z_ext, out_ext, zs, z_done, zoff, foff,
                vsem, asem, osem, fcs=None):
    """Like _build_dve but input chunks alternate between the two HWDGE
    rings (even -> SP/qSP, odd -> ACT/qAct) to test whether one ring caps
    the streaming bandwidth."""
    def v_sub_done(r, ch):
        return 3 * nch * r + 3 * ch + 1

    def v_red_done(r, ch):
        return 3 * nch * r + 3 * ch + 3

    def issue_dma(eng, r, ch):
        if r >= nbuf:
            eng.wait_ge(vsem, v_sub_done(r - nbuf, ch))
        eng.dma_start(
            out=zt[:, zoff(r) + ch * 2 * fc:zoff(r) + (ch + 1) * 2 * fc],
            in_=z_ext[:, ch * 2 * fc:(ch + 1) * 2 * fc],
        ).then_inc(zs(r, ch), 16)

    @block.sync
    def _(sync):
        for r in range(repeat):
            for ch in range(0, nch, 2):
                issue_dma(sync, r, ch)

    @block.vector
    def _(vector):
        for r in range(repeat):
            if r >= nbuf:
                vector.wait_ge(asem, r - nbuf + 1)
            for ch in range(nch):
                vector.wait_ge(zs(r, ch), z_done(r))
                lo, hi = foff(r) + ch * fc, foff(r) + (ch + 1) * fc
                vector.tensor_sub(
                    df[:, lo:hi],
                    zt[:, zoff(r) + ch * 2 * fc:zoff(r) + ch * 2 * fc + fc],
                    zt[:, zoff(r) + ch * 2 * fc + fc:
                       zoff(r) + (ch + 1) * 2 * fc],
                ).then_inc(vsem, 1)
                vector.tensor_mul(
                    sq[:, lo:hi], df[:, lo:hi], df[:, lo:hi]
                ).then_inc(vsem, 1)
                vector.tensor_reduce(
                    out=gs[:, (r % nbuf) * GT + ch * gpc:
                           (r % nbuf) * GT + (ch + 1) * gpc],
                    in_=sq[:, lo:hi].rearrange("p (g c) -> p g c", c=C),
                    axis=mybir.AxisListType.X,
                    op=mybir.AluOpType.add,
                ).then_inc(vsem, 1)

    @block.scalar
    def _(scalar):
        def sqrt_rep(r):
            scalar.wait_ge(vsem, v_red_done(r, nch - 1))
            scalar.activation(
                out=dist[:, (r % nbuf) * GT:(r % nbuf + 1) * GT],
                in_=gs[:, (r % nbuf) * GT:(r % nbuf + 1) * GT],
                func=mybir.ActivationFunctionType.Sqrt,
                accum_out=acc[:, r % nbuf:r % nbuf + 1],
            ).then_inc(asem, 1)

        # software-pipelined by one rep so ACT's DMA issues for rep r are
        # enqueued before sqrt(r-1) blocks the sequencer (a deadlock-free
        # order w.r.t. DVE's gs-WAR wait on asem)
        for r in range(repeat):
            for ch in range(1, nch, 2):
                issue_dma(scalar, r, ch)
            if r >= 1:
                sqrt_rep(r - 1)
        sqrt_rep(repeat - 1)
        scalar.wait_ge(asem, repeat)
        scalar.dma_start(
            out=out_ext[:],
            in_=acc[:, (repeat - 1) % nbuf:(repeat - 1) % nbuf + 1],
        ).then_inc(osem, 16)
        scalar.wait_ge(osem, 16)


def _build_dve(nc, block, repeat, nch, fc, gpc, nbuf,
               zt, df, sq, gs, dist, acc,
               z_ext, out_ext, zs, z_done, zoff, foff,
               vsem, asem, osem, fcs=None):
    # vsem: 3 DVE ops per chunk (sub, square, reduce)
    def v_sub_done(r, ch):
        return 3 * nch * r + 3 * ch + 1

    def v_red_done(r, ch):
        return 3 * nch * r + 3 * ch + 3

    @block.sync
    def _(sync):
        for r in range(repeat):
            for ch in range(nch):
                if r >= nbuf:
                    # WAR: rep r reuses rep r-nbuf's zt chunk; its sub
                    # must have consumed it
                    sync.wait_ge(vsem, v_sub_done(r - nbuf, ch))
                sync.dma_start(
                    out=zt[:, zoff(r) + ch * 2 * fc:
                           zoff(r) + (ch + 1) * 2 * fc],
                    in_=z_ext[:, ch * 2 * fc:(ch + 1) * 2 * fc],
                ).then_inc(zs(r, ch), 16)

    @block.vector
    def _(vector):
        for r in range(repeat):
            if r >= nbuf:
                # WAR: gs slot r%nbuf was read by sqrt of rep r-nbuf
                vector.wait_ge(asem, r - nbuf + 1)
            for ch in range(nch):
                vector.wait_ge(zs(r, ch), z_done(r))
                lo, hi = foff(r) + ch * fc, foff(r) + (ch + 1) * fc
                vector.tensor_sub(
                    df[:, lo:hi],
                    zt[:, zoff(r) + ch * 2 * fc:zoff(r) + ch * 2 * fc + fc],
                    zt[:, zoff(r) + ch * 2 * fc + fc:
                       zoff(r) + (ch + 1) * 2 * fc],
                ).then_inc(vsem, 1)
                vector.tensor_mul(
                    sq[:, lo:hi], df[:, lo:hi], df[:, lo:hi]
                ).then_inc(vsem, 1)
                vector.tensor_reduce(
                    out=gs[:, (r % nbuf) * GT + ch * gpc:
                           (r % nbuf) * GT + (ch + 1) * gpc],
                    in_=sq[:, lo:hi].rearrange("p (g c) -> p g c", c=C),
                    axis=mybir.AxisListType.X,
                    op=mybir.AluOpType.add,
                ).then_inc(vsem, 1)

    @block.scalar
    def _(scalar):
        for r in range(repeat):
            scalar.wait_ge(vsem, v_red_done(r, nch - 1))
            scalar.activation(
                out=dist[:, (r % nbuf) * GT:(r % nbuf + 1) * GT],
                in_=gs[:, (r % nbuf) * GT:(r % nbuf + 1) * GT],
                func=mybir.ActivationFunctionType.Sqrt,
                accum_out=acc[:, r % nbuf:r % nbuf + 1],
            ).then_inc(asem, 1)
        # self-wait: the HWDGE dma_start below reads acc written by the
        # activation above; the sequencer runs ahead of the compute pipe,
        # so order must be enforced via the sem
        scalar.wait_ge(asem, repeat)
        scalar.dma_start(
            out=out_ext[:],
            in_=acc[:, (repeat - 1) % nbuf:(repeat - 1) % nbuf + 1],
        ).then_inc(osem, 16)
        scalar.wait_ge(osem, 16)


def _build_hyb(nc, block, repeat, nch, fc, gpc, nbuf,
               zt, df, sq, gs, dist, acc,
               z_ext, out_ext, zs, z_done, zoff, foff,
               vsem, asem, osem, fcs=None):
    """Streaming chunks 0..nch-2 use the act layout (DVE sub -> ACT square
    -> DVE reduce: DVE stays at 2 passes, under the DMA stream). The LAST
    chunk runs sub/square/reduce entirely on DVE in program order, removing
    both cross-engine hops from the critical tail. ACT then does sqrt(+acc)
    and issues the output DMA from its own ring."""
    VR = 2 * nch + 1   # vsem ticks per rep
    AR = nch           # asem ticks per rep: nch-1 squares + 1 sqrt
    if fcs is None:
        fcs = [fc] * nch
    fo = [0]
    for s in fcs:
        fo.append(fo[-1] + s)   # prefix offsets (elems/partition)

    # DVE program order per rep: sub0, [sub1, red0], ..., [sub_{n-2},
    # red_{n-3}], red_{n-2}, sub_{n-1}, mul_last, red_last — reduces
    # interleave into the DMA-bound stream, and red_{n-2} is hoisted before
    # the last sub (its square lands ~a chunk before the final DMA does) so
    # only the last chunk's three DVE ops remain after the stream ends
    def v_sub_done(r, ch):
        if ch == 0:
            return VR * r + 1
        if ch == nch - 1:
            return VR * r + 2 * nch - 1
        return VR * r + 2 * ch

    def v_red_done(r, ch):   # ch <= nch-2
        return VR * r + (2 * nch - 2 if ch == nch - 2 else 2 * ch + 3)

    def v_red_last_done(r):
        return VR * (r + 1)

    def a_sq_done(r, ch):    # ch <= nch-2
        return AR * r + ch + 1

    def a_sqrt_done(r):
        return AR * (r + 1)

    @block.sync
    def _(sync):
        for r in range(repeat):
            for ch in range(nch):
                if r >= nbuf:
                    # WAR: rep r reuses rep r-nbuf's zt chunk
                    sync.wait_ge(vsem, v_sub_done(r - nbuf, ch))
                sync.dma_start(
                    out=zt[:, zoff(r) + 2 * fo[ch]:
                           zoff(r) + 2 * fo[ch + 1]],
                    in_=z_ext[:, 2 * fo[ch]:2 * fo[ch + 1]],
                ).then_inc(zs(r, ch), 16)

    @block.vector
    def _(vector):
        def sub(r, ch):
            lo, hi = foff(r) + fo[ch], foff(r) + fo[ch + 1]
            zb = zoff(r) + 2 * fo[ch]
            return vector.tensor_sub(
                df[:, lo:hi],
                zt[:, zb:zb + fcs[ch]],
                zt[:, zb + fcs[ch]:zb + 2 * fcs[ch]],
            ).then_inc(vsem, 1)

        def red(r, ch):
            lo, hi = foff(r) + fo[ch], foff(r) + fo[ch + 1]
            return vector.tensor_reduce(
                out=gs[:, (r % nbuf) * GT + fo[ch] // C:
                       (r % nbuf) * GT + fo[ch + 1] // C],
                in_=sq[:, lo:hi].rearrange("p (g c) -> p g c", c=C),
                axis=mybir.AxisListType.X,
                op=mybir.AluOpType.add,
            ).then_inc(vsem, 1)

        for r in range(repeat):
            for ch in range(nch - 1):
                vector.wait_ge(zs(r, ch), z_done(r))
                if r >= nbuf:
                    # WAR: df slot last read by square(r-nbuf, ch); the last
                    # chunk's df is read by mul(r-nbuf) on this engine
                    vector.wait_ge(asem, a_sq_done(r - nbuf, ch))
                sub(r, ch)
                if ch >= 1:
                    # red of the previous chunk; its square is usually done
                    # by the time this chunk's DMA has landed
                    vector.wait_ge(asem, a_sq_done(r, ch - 1))
                    red(r, ch - 1)
            # hoist the second-to-last reduce ahead of the final sub
            vector.wait_ge(asem, a_sq_done(r, nch - 2))
            red(r, nch - 2)
            ch = nch - 1
            vector.wait_ge(zs(r, ch), z_done(r))
            sub(r, ch)
            lo, hi = foff(r) + fo[ch], foff(r) + fo[ch + 1]
            vector.tensor_mul(
                sq[:, lo:hi], df[:, lo:hi], df[:, lo:hi]
            ).then_inc(vsem, 1)
            red(r, ch)

    @block.scalar
    def _(scalar):
        for r in range(repeat):
            for ch in range(nch - 1):
                scalar.wait_ge(vsem, v_sub_done(r, ch))
                scalar.square(
                    out=sq[:, foff(r) + fo[ch]:foff(r) + fo[ch + 1]],
                    in_=df[:, foff(r) + fo[ch]:foff(r) + fo[ch + 1]],
                ).then_inc(asem, 1)
            scalar.wait_ge(vsem, v_red_last_done(r))
            scalar.activation(
                out=dist[:, (r % nbuf) * GT:(r % nbuf + 1) * GT],
                in_=gs[:, (r % nbuf) * GT:(r % nbuf + 1) * GT],
                func=mybir.ActivationFunctionType.Sqrt,
                accum_out=acc[:, r % nbuf:r % nbuf + 1],
            ).then_inc(asem, 1)
        # self-wait before reading our own activation's output via HWDGE
        scalar.wait_ge(asem, a_sqrt_done(repeat - 1))
        scalar.dma_start(
            out=out_ext[:],
            in_=acc[:, (repeat - 1) % nbuf:(repeat - 1) % nbuf + 1],
        ).then_inc(osem, 16)
        scalar.wait_ge(osem, 16)


def _build_hyd(nc, block, repeat, nch, fc, gpc, nbuf,
               zt, df, sq, gs, dist, acc,
               z_ext, out_ext, zs, z_done, zoff, foff,
               vsem, asem, osem, fcs=None):
    """_build_hyb with input DMAs alternating between the two HWDGE rings:
    even chunks from SP (qSP), odd chunks from ACT (qAct), to test whether a
    single ring caps streaming bandwidth. DVE block is identical to hyb."""
    VR = 2 * nch + 1
    AR = nch

    def v_sub_done(r, ch):
        return VR * r + (1 if ch == 0 else 2 * ch)

    def v_red_last_done(r):
        return VR * (r + 1)

    def a_sq_done(r, ch):    # ch <= nch-2
        return AR * r + ch + 1

    def a_sqrt_done(r):
        return AR * (r + 1)

    def issue_dma(eng, r, ch):
        if r >= nbuf:
            eng.wait_ge(vsem, v_sub_done(r - nbuf, ch))
        eng.dma_start(
            out=zt[:, zoff(r) + ch * 2 * fc:zoff(r) + (ch + 1) * 2 * fc],
            in_=z_ext[:, ch * 2 * fc:(ch + 1) * 2 * fc],
        ).then_inc(zs(r, ch), 16)

    @block.sync
    def _(sync):
        for r in range(repeat):
            for ch in range(0, nch, 2):
                issue_dma(sync, r, ch)

    @block.vector
    def _(vector):
        def sub(r, ch):
            lo, hi = foff(r) + ch * fc, foff(r) + (ch + 1) * fc
            return vector.tensor_sub(
                df[:, lo:hi],
                zt[:, zoff(r) + ch * 2 * fc:zoff(r) + ch * 2 * fc + fc],
                zt[:, zoff(r) + ch * 2 * fc + fc:zoff(r) + (ch + 1) * 2 * fc],
            ).then_inc(vsem, 1)

        def red(r, ch):
            lo, hi = foff(r) + ch * fc, foff(r) + (ch + 1) * fc
            return vector.tensor_reduce(
                out=gs[:, (r % nbuf) * GT + ch * gpc:
                       (r % nbuf) * GT + (ch + 1) * gpc],
                in_=sq[:, lo:hi].rearrange("p (g c) -> p g c", c=C),
                axis=mybir.AxisListType.X,
                op=mybir.AluOpType.add,
            ).then_inc(vsem, 1)

        for r in range(repeat):
            for ch in range(nch):
                vector.wait_ge(zs(r, ch), z_done(r))
                if r >= nbuf and ch < nch - 1:
                    vector.wait_ge(asem, a_sq_done(r - nbuf, ch))
                sub(r, ch)
                if ch >= 1:
                    vector.wait_ge(asem, a_sq_done(r, ch - 1))
                    red(r, ch - 1)
            ch = nch - 1
            lo, hi = foff(r) + ch * fc, foff(r) + (ch + 1) * fc
            vector.tensor_mul(
                sq[:, lo:hi], df[:, lo:hi], df[:, lo:hi]
            ).then_inc(vsem, 1)
            red(r, ch)

    @block.scalar
    def _(scalar):
        def squares_and_sqrt(r):
            for ch in range(nch - 1):
                scalar.wait_ge(vsem, v_sub_done(r, ch))
                scalar.square(
                    out=sq[:, foff(r) + ch * fc:foff(r) + (ch + 1) * fc],
                    in_=df[:, foff(r) + ch * fc:foff(r) + (ch + 1) * fc],
                ).then_inc(asem, 1)
            scalar.wait_ge(vsem, v_red_last_done(r))
            scalar.activation(
                out=dist[:, (r % nbuf) * GT:(r % nbuf + 1) * GT],
                in_=gs[:, (r % nbuf) * GT:(r % nbuf + 1) * GT],
                func=mybir.ActivationFunctionType.Sqrt,
                accum_out=acc[:, r % nbuf:r % nbuf + 1],
            ).then_inc(asem, 1)

        # odd-chunk DMA issues lead their rep's compute by one iteration so
        # squares/sqrt waits never park the ring behind stale work
        for r in range(repeat):
            for ch in range(1, nch, 2):
                issue_dma(scalar, r, ch)
            if r >= 1:
                squares_and_sqrt(r - 1)
        squares_and_sqrt(repeat - 1)
        scalar.wait_ge(asem, a_sqrt_done(repeat - 1))
        scalar.dma_start(
            out=out_ext[:],
            in_=acc[:, (repeat - 1) % nbuf:(repeat - 1) % nbuf + 1],
        ).then_inc(osem, 16)
        scalar.wait_ge(osem, 16)


def _build_act(nc, block, repeat, nch, fc, gpc, nbuf,
               zt, df, sq, gs, dist, acc,
               z_ext, out_ext, zs, z_done, zoff, foff,
               vsem, asem, osem, fcs=None):
    # vsem: per rep, nch subs then nch reduces
    def v_sub_done(r, ch):
        return 2 * nch * r + ch + 1

    def v_red_done(r, ch):
        return 2 * nch * r + nch + ch + 1

    # asem: per rep, nch squares then one sqrt
    def a_sq_done(r, ch):
        return (nch + 1) * r + ch + 1

    def a_sqrt_done(r):
        return (nch + 1) * (r + 1)

    @block.sync
    def _(sync):
        for r in range(repeat):
            for ch in range(nch):
                if r >= nbuf:
                    sync.wait_ge(vsem, v_sub_done(r - nbuf, ch))
                sync.dma_start(
                    out=zt[:, zoff(r) + ch * 2 * fc:
                           zoff(r) + (ch + 1) * 2 * fc],
                    in_=z_ext[:, ch * 2 * fc:(ch + 1) * 2 * fc],
                ).then_inc(zs(r, ch), 16)

    @block.vector
    def _(vector):
        for r in range(repeat):
            for ch in range(nch):
                vector.wait_ge(zs(r, ch), z_done(r))
                if r >= nbuf:
                    # WAR: df slot last read by square(r-nbuf, ch)
                    vector.wait_ge(asem, a_sq_done(r - nbuf, ch))
                vector.tensor_sub(
                    df[:, foff(r) + ch * fc:foff(r) + (ch + 1) * fc],
                    zt[:, zoff(r) + ch * 2 * fc:zoff(r) + ch * 2 * fc + fc],
                    zt[:, zoff(r) + ch * 2 * fc + fc:
                       zoff(r) + (ch + 1) * 2 * fc],
                ).then_inc(vsem, 1)
            for ch in range(nch):
                vector.wait_ge(asem, a_sq_done(r, ch))
                vector.tensor_reduce(
                    out=gs[:, (r % nbuf) * GT + ch * gpc:
                           (r % nbuf) * GT + (ch + 1) * gpc],
                    in_=sq[:, foff(r) + ch * fc:foff(r) + (ch + 1) * fc]
                    .rearrange("p (g c) -> p g c", c=C),
                    axis=mybir.AxisListType.X,
                    op=mybir.AluOpType.add,
                ).then_inc(vsem, 1)

    @block.scalar
    def _(scalar):
        for r in range(repeat):
            for ch in range(nch):
                # sub(r, ch) done also implies the rep r-nbuf reduce that
                # last read this sq slot finished (WAR safe)
                scalar.wait_ge(vsem, v_sub_done(r, ch))
                scalar.square(
                    out=sq[:, foff(r) + ch * fc:foff(r) + (ch + 1) * fc],
                    in_=df[:, foff(r) + ch * fc:foff(r) + (ch + 1) * fc],
                ).then_inc(asem, 1)
            scalar.wait_ge(vsem, v_red_done(r, nch - 1))
            scalar.activation(
                out=dist[:, (r % nbuf) * GT:(r % nbuf + 1) * GT],
                in_=gs[:, (r % nbuf) * GT:(r % nbuf + 1) * GT],
                func=mybir.ActivationFunctionType.Sqrt,
                accum_out=acc[:, r % nbuf:r % nbuf + 1],
            ).then_inc(asem, 1)
        # self-wait before reading our own activation's output via HWDGE
        scalar.wait_ge(asem, a_sqrt_done(repeat - 1))
        scalar.dma_start(
            out=out_ext[:],
            in_=acc[:, (repeat - 1) % nbuf:(repeat - 1) % nbuf + 1],
        ).then_inc(osem, 16)
        scalar.wait_ge(osem, 16)


def pack_inputs(X, Y, nch=None):
    """(B,T,N,C) x2 -> per-core packed z arrays, chunk-interleaved x|y.

    nch=None (graded default) uses the CHUNK_SIZES vector (must match
    _build's default); an explicit nch gives uniform chunks (bench paths).
    """
    fcs = list(CHUNK_SIZES) if nch is None else [F // nch] * nch
    X = np.asarray(X, dtype=np.float32).reshape(N_CORES, P, F)
    Y = np.asarray(Y, dtype=np.float32).reshape(N_CORES, P, F)
    Z = np.empty((N_CORES, P, 2 * F), dtype=np.float32)
    o = 0
    for s in fcs:
        Z[:, :, 2 * o:2 * o + s] = X[:, :, o:o + s]
        Z[:, :, 2 * o + s:2 * o + 2 * s] = Y[:, :, o:o + s]
        o += s
    return Z


def kernel(X, Y, window=None, **_):
    global _nc_cache
    Z = pack_inputs(X, Y)
    if _nc_cache is None:
        _nc_cache = _build()
    in_maps = [{"z": Z[k]} for k in range(N_CORES)]
    res = run_bass_kernel_spmd(_nc_cache, in_maps, list(range(N_CORES)))
    global _last_results
    _last_results = res
    partials = np.stack([r["out"] for r in res.results])  # (8, 128, 1)
    total = partials.astype(np.float64).sum()
    return np.float32(total / (B * N))

